# revision 16
# baseline (speedup 1.0000x reference)
"""Location-aware attention on 8 Trainium2 NeuronCores (Bass/Tile).

Math (per batch item, reference semantics):
    enc_h = enc @ W_enc + b_enc                      [T, A]
    prev  = mask/L  (uniform attention over valid positions)
    loc   = conv1d(prev, conv_w).T @ W_loc + b_loc   [T, A]
    e     = tanh(enc_h + dec_h + loc) @ w_attn       [T]
    attn  = softmax(mask ? e : -inf)                 [T]
    ctx   = (attn @ enc) @ W_out + b_out             [O]

Kernel strategy (data-parallel over batch, 4 items/core):
  * prev is a step function, so conv1d(prev) is CONSTANT in t except within
    K=100 of the ends: loc(t) = PW[201]/L for t in [K, L-1-K], where PW is
    the prefix-sum over taps of Wj = conv_w^T @ W_loc (a host weight
    transform). The interior folds into the per-item tanh bias (together
    with dec_h and all biases); the head (t<K, static) and tail
    (t in [L-K, L), depends on L) get exact small correction tables added
    to the PSUM before the tanh.
  * enc_len is known when the kernel traces, so the program is specialized:
    items are L-sorted and dealt round-robin (core c, slot j <- rank j*8+c),
    which balances cores AND makes each slot's supertile count/edge windows
    near-uniform; work for t-tiles beyond a slot's max L is skipped.
  * The big GEMM runs in bf16 (measured 2.4e-3 rel err at K=512) in the
    [a-partitions, t-free] orientation: lhsT = W_enc k-tiles (native
    layout), rhs = enc^T built on-chip by PE transposes of the bf16 copy;
    enc itself is cast f32->bf16 inside the load DMA (SWDGE cast).
  * e = w_attn^T @ tanh via 1-column-stationary matvecs; e rows are staged
    through DRAM (partition-crossing move without SBUF->SBUF DMAs, which
    would serialize against the rest); masked softmax runs once on [4, T].
  * The bf16 enc copy stays resident in SBUF, so the context matvec needs
    no second HBM read; attention columns come from one xbar DMA transpose.
"""
import os

import numpy as np

B, T, ENC, DEC, ATT, CCH, OUT, K = 32, 2000, 512, 512, 512, 32, 512, 100
N_CORES = 8
BPC = B // N_CORES
SUPER = [(0, 512), (512, 512), (1024, 512), (1536, 464)]

_built = {}


# ---------------------------------------------------------------------------
# Walrus compat: this container's walrus build accepts at most ONE semaphore
# wait per instruction. (a) split the Tile exit-drain waits across several
# drains; (b) a to_json_bytes post-pass hoists excess waits onto
# EventSemaphore instructions inserted directly before the owner (same
# engine, so it blocks on the same condition one instruction earlier).
# ---------------------------------------------------------------------------
def _apply_walrus_patches():
    import orjson

    import concourse.bass as bass
    import concourse.mybir as mybir
    import concourse.tile as tile
    from concourse.vector_clock import ScopedClock

    if getattr(tile.TileContext, "_onewait_patched", False):
        return

    def _drain_and_barrier(self, tick_clock, wait_clock):
        nc = self.nc
        drain_inst = nc.sync.drain()
        wait_clock.add_sem_waits(
            drain_inst.ins, ScopedClock({None: tick_clock.global_clock})
        )
        ins = drain_inst.ins
        si = ins.sync_info
        waits = list(si.on_wait) if si and si.on_wait else []
        if len(waits) > 1:
            ins.sync_info = mybir.SyncInfo(
                on_wait=waits[:1], on_update=list(si.on_update or [])
            )
            for w in waits[1:]:
                d2 = nc.sync.drain()
                d2.ins.sync_info = mybir.SyncInfo(on_wait=[w], on_update=[])
        nc.all_engine_barrier()
        assert self.sems is not None
        popped = nc._tile_sem_poison_stack.pop()
        assert popped is self._sem_poison
        nc.clear_and_free_semaphores(list(self.sems.allocated().values()))
        nc.all_engine_barrier()

    _orig_to_json_bytes = bass.Bass.to_json_bytes
    counter = [0]

    def _split_to_json_bytes(self) -> bytes:
        j = orjson.loads(_orig_to_json_bytes(self))
        changed = False
        for f in j["functions"]:
            for bb in f["blocks"]:
                insts = bb.get("instructions") or []
                out = []
                for inst in insts:
                    si = inst.get("sync_info")
                    waits = (si or {}).get("on_wait") or []
                    if len(waits) > 1:
                        changed = True
                        for w in waits[:-1]:
                            counter[0] += 1
                            out.append({
                                "debug": inst.get("debug", 0),
                                "engine": inst["engine"],
                                "ins": [],
                                "outs": [],
                                "name": f"I-wsplit-{counter[0]}",
                                "opcode": "EventSemaphore",
                                "sync_info": {"on_update": [], "on_wait": [w]},
                            })
                        si["on_wait"] = [waits[-1]]
                    out.append(inst)
                bb["instructions"] = out
        return orjson.dumps(j) if changed else _orig_to_json_bytes(self)

    tile.TileContext._drain_and_barrier = _drain_and_barrier
    bass.Bass.to_json_bytes = _split_to_json_bytes
    tile.TileContext._onewait_patched = True


def _build_nc(spec):
    """spec: dict with static per-slot specialization:
    nsb[j]   - number of supertiles to compute for slot j
    tw0[j]   - tail-correction window start (col offset into ctail input)
    twW[j]   - tail window width (<= Wmax)
    Wmax     - ctail last-dim size
    """
    import concourse.bass as bass
    import concourse.mybir as mybir
    import concourse.tile as tile
    from concourse.masks import make_identity

    dt = mybir.dt
    f32, f32r, bf16, i32 = dt.float32, dt.float32r, dt.bfloat16, dt.int32
    Alu = mybir.AluOpType
    Act = mybir.ActivationFunctionType
    Axis = mybir.AxisListType

    nsb = spec["nsb"]
    tw0 = spec["tw0"]
    twW = spec["twW"]
    Wmax = spec["Wmax"]

    nc = bass.Bass()

    enc = nc.dram_tensor("enc", [BPC, T, ENC], f32, kind="ExternalInput")
    encl4 = nc.dram_tensor("encl4", [BPC, 1], i32, kind="ExternalInput")
    wenc = nc.dram_tensor("wenc", [128, 4, ATT], f32, kind="ExternalInput")
    wdec = nc.dram_tensor("wdec", [128, 4, ATT], f32, kind="ExternalInput")
    wout = nc.dram_tensor("wout", [128, 4, OUT], f32, kind="ExternalInput")
    wat4 = nc.dram_tensor("wat4", [128, 4], f32, kind="ExternalInput")
    dec4 = nc.dram_tensor("dec4", [128, 4, BPC], f32, kind="ExternalInput")
    bias4 = nc.dram_tensor("bias4", [128, 4, BPC], f32, kind="ExternalInput")
    chead = nc.dram_tensor("chead", [BPC, 128, 4, K], bf16, kind="ExternalInput")
    ctail = nc.dram_tensor("ctail", [BPC, 128, 4, Wmax], bf16,
                           kind="ExternalInput")
    boutd = nc.dram_tensor("boutd", [1, OUT], f32, kind="ExternalInput")
    e_dram = nc.dram_tensor("e_dram", [BPC, T], f32)
    ng_dram = nc.dram_tensor("ng_dram", [1, 1], f32)
    ctx_o = nc.dram_tensor("ctx_o", [BPC, OUT], f32, kind="ExternalOutput")
    attn_o = nc.dram_tensor("attn_o", [BPC, T], f32, kind="ExternalOutput")

    with tile.TileContext(nc) as tc:
        with tc.tile_pool(name="const", bufs=1) as const, \
             tc.tile_pool(name="ebuf", bufs=1) as ebuf, \
             tc.tile_pool(name="et", bufs=3) as et_pool, \
             tc.tile_pool(name="th", bufs=6) as th_pool, \
             tc.tile_pool(name="ww", bufs=2) as ww_pool, \
             tc.tile_pool(name="fin", bufs=1) as fin, \
             tc.tile_pool(name="tr", bufs=3, space="PSUM") as tr_ps, \
             tc.tile_pool(name="xp", bufs=2, space="PSUM") as x_ps, \
             tc.tile_pool(name="mi", bufs=3, space="PSUM") as mi_ps:

            # ---------------- constants / weights -----------------
            identity_bf = const.tile([128, 128], bf16)
            make_identity(nc, identity_bf)
            identity = const.tile([128, 128], f32)
            make_identity(nc, identity)

            wenc_b = const.tile([128, 4, ATT], bf16)
            wout_b = const.tile([128, 4, OUT], bf16)
            wat4_b = const.tile([128, 4], bf16)
            base4 = const.tile([128, 4, BPC], f32)  # tanh bias per (a,m,item)
            bout_s = const.tile([1, OUT], f32)
            e_sb = const.tile([BPC, T], f32)
            gmax = const.tile([1, 512], f32)
            negmx4 = const.tile([BPC, 1], f32)
            elf = const.tile([BPC, 1], f32)
            ch_t = [const.tile([128, 4, K], bf16, name=f"ch_{j}")
                    for j in range(BPC)]
            ct_t = [const.tile([128, 4, Wmax], bf16, name=f"ct_{j}")
                    for j in range(BPC)]
            nc.vector.memset(e_sb, -30.0)
            nc.vector.memset(gmax, -1e30)
            # per-item edge-correction tables (a-major packing)
            for j in range(BPC):
                nc.sync.dma_start(out=ch_t[j], in_=chead[j, :, :, :])
                nc.sync.dma_start(out=ct_t[j], in_=ctail[j, :, :, :])

            nc.gpsimd.dma_start(out=wenc_b, in_=wenc[:, :, :])
            nc.gpsimd.dma_start(out=wout_b, in_=wout[:, :, :])
            nc.gpsimd.dma_start(out=wat4_b, in_=wat4[:, :])
            with tc.tile_pool(name="stage", bufs=1) as stage:
                nc.sync.dma_start(out=bout_s, in_=boutd[:, :])

                # base4[:, m, item] = dec_h^T + bias4 (bias4 already folds
                # b_enc+b_dec+b_loc and the interior conv response PW/L)
                s_wdec = stage.tile([128, 4, ATT], f32, tag="s8", name="s_wdec")
                nc.sync.dma_start(out=s_wdec, in_=wdec[:, :, :])
                s_dec4 = stage.tile([128, 4, BPC], f32, tag="s0b")
                nc.sync.dma_start(out=s_dec4, in_=dec4[:, :, :])
                s_bias4 = stage.tile([128, 4, BPC], f32, tag="s0c")
                nc.sync.dma_start(out=s_bias4, in_=bias4[:, :, :])
                for m in range(4):
                    ps_d = mi_ps.tile([128, BPC], f32, tag="mi")
                    for kd in range(4):
                        nc.tensor.matmul(
                            ps_d,
                            s_wdec[:, kd, m * 128:(m + 1) * 128],
                            s_dec4[:, kd, :],
                            start=(kd == 0), stop=(kd == 3),
                        )
                    nc.vector.tensor_add(base4[:, m, :], ps_d, s_bias4[:, m, :])

                s_el = stage.tile([BPC, 1], i32, tag="s0d")
                nc.sync.dma_start(out=s_el, in_=encl4[:, :])
                nc.vector.tensor_copy(elf, s_el)

            iota_f = fin.tile([BPC, T], f32, tag="iotaf")
            maskf = fin.tile([BPC, T], f32, tag="maskf")
            nc.gpsimd.iota(iota_f, pattern=[[1, T]], base=0,
                           channel_multiplier=0,
                           allow_small_or_imprecise_dtypes=True)
            nc.vector.tensor_scalar(
                out=maskf, in0=iota_f, scalar1=elf, scalar2=None, op0=Alu.is_lt)

            # ---------------- persistent bf16 enc copies -----------------
            ebf = {}
            for j in range(BPC):
                for si in range(nsb[j]):
                    ebf[(j, si)] = ebuf.tile(
                        [128, 4, ENC], bf16, tag=f"ebf_{j}_{si}",
                        name=f"ebf_{j}_{si}")

            def emit_transposes(j, si, et):
                t0, Nt = SUPER[si]
                for tb in range((Nt + 127) // 128):
                    ptb = min(128, Nt - tb * 128)
                    for ke in range(4):
                        pst = tr_ps.tile([128, 128], bf16, tag="tr")
                        nc.tensor.transpose(
                            pst[:, :ptb],
                            ebf[(j, si)][:ptb, tb, ke * 128:(ke + 1) * 128],
                            identity_bf[:ptb, :ptb])
                        nc.vector.tensor_copy(
                            et[:, ke, tb * 128: tb * 128 + ptb], pst[:, :ptb])

            # work list: (j, si) in supertile-major order
            work = [(j, si) for si in range(4) for j in range(BPC)
                    if si < nsb[j]]

            # batched cast-loads (SWDGE casts f32->bf16 during transfer)
            for si in range(4):
                for j in range(BPC):
                    if si >= nsb[j]:
                        continue
                    t0, Nt = SUPER[si]
                    for tb in range((Nt + 127) // 128):
                        ptb = min(128, Nt - tb * 128)
                        nc.gpsimd.dma_start(
                            out=ebf[(j, si)][:ptb, tb, :],
                            in_=enc[j, t0 + tb * 128: t0 + tb * 128 + ptb, :])

            # pipeline: transposes for the next work item are emitted before
            # this item's e-matvec so the PE never waits on the last tanh
            ets = {}
            ets[0] = et_pool.tile([128, 4, 512], bf16, tag="et", name="et_p0")
            emit_transposes(*work[0], ets[0])
            for w, (j, si) in enumerate(work):
                t0, Nt = SUPER[si]
                et = ets.pop(w)
                # ---- fused GEMM: enc_h^T (+ edge corrections), tanh ----
                ths = []
                xs = []
                for m in range(4):
                    xps = x_ps.tile([128, 512], f32, tag="x")
                    for ke in range(4):
                        nc.tensor.matmul(
                            xps[:, :Nt],
                            wenc_b[:, ke, m * 128:(m + 1) * 128],
                            et[:, ke, :Nt],
                            start=(ke == 0), stop=(ke == 3))
                    xs.append(xps)
                # edge corrections into PSUM (DVE), before tanh
                for m in range(4):
                    if si == 0:
                        nc.vector.tensor_add(
                            xs[m][:, :K], xs[m][:, :K], ch_t[j][:, m, :])
                    c0 = max(t0, tw0[j])
                    c1 = min(t0 + Nt, tw0[j] + twW[j])
                    if c0 < c1:
                        nc.vector.tensor_add(
                            xs[m][:, c0 - t0: c1 - t0],
                            xs[m][:, c0 - t0: c1 - t0],
                            ct_t[j][:, m, c0 - tw0[j]: c1 - tw0[j]])
                for m in range(4):
                    th = th_pool.tile([128, 512], bf16, tag="th")
                    nc.scalar.activation(
                        out=th[:, :Nt], in_=xs[m][:, :Nt], func=Act.Tanh,
                        bias=base4[:, m, j:j + 1], scale=1.0)
                    ths.append(th)
                # emit next work item's transposes ahead of the e-matvec
                if w + 1 < len(work):
                    ets[w + 1] = et_pool.tile([128, 4, 512], bf16, tag="et",
                                              name=f"et_p{w+1}")
                    emit_transposes(*work[w + 1], ets[w + 1])
                # ---- e = w_attn^T @ tanh -> [1, Nt] ----
                pe = mi_ps.tile([1, 512], f32, tag="mi")
                for m in range(4):
                    nc.tensor.matmul(
                        pe[:, :Nt], wat4_b[:, m:m + 1], ths[m][:, :Nt],
                        start=(m == 0), stop=(m == 3))
                etmp = ww_pool.tile([1, 512], f32, tag="etmp")
                nc.vector.tensor_copy(etmp[:, :Nt], pe[:, :Nt])
                nc.vector.tensor_max(
                    gmax[:, :Nt], gmax[:, :Nt], etmp[:, :Nt])
                # partition-crossing move staged via DRAM (no SBUF->SBUF DMA)
                nc.scalar.dma_start(
                    out=e_dram[j:j + 1, t0:t0 + Nt], in_=etmp[:, :Nt])

            # ---------------- masked softmax on [4, T] -----------------
            for j in range(BPC):
                end_j = SUPER[nsb[j] - 1][0] + SUPER[nsb[j] - 1][1]
                nc.scalar.dma_start(
                    out=e_sb[j:j + 1, :end_j], in_=e_dram[j:j + 1, :end_j])
            g1 = fin.tile([1, 1], f32, tag="g1")
            nc.vector.tensor_reduce(out=g1, in_=gmax, axis=Axis.X, op=Alu.max)
            ng1 = fin.tile([1, 1], f32, tag="ng1")
            nc.vector.tensor_scalar_mul(ng1, g1, -1.0)
            nc.scalar.dma_start(out=ng_dram[:, :], in_=ng1)
            nc.scalar.dma_start(out=negmx4, in_=bass.AP(
                ng_dram[:, :].tensor, 0, [[0, BPC], [1, 1]]))
            p_sb = fin.tile([BPC, T], f32, tag="p")
            nc.scalar.activation(
                out=p_sb, in_=e_sb, func=Act.Exp, bias=negmx4, scale=1.0)
            nc.vector.tensor_mul(p_sb, p_sb, maskf)
            ssum = fin.tile([BPC, 1], f32, tag="ssum")
            nc.vector.tensor_reduce(out=ssum, in_=p_sb, axis=Axis.X, op=Alu.add)
            rs = fin.tile([BPC, 1], f32, tag="rs")
            nc.vector.reciprocal(rs, ssum)
            nc.vector.tensor_scalar(
                out=p_sb, in0=p_sb, scalar1=rs, scalar2=None, op0=Alu.mult)
            attn_sb = p_sb
            nc.sync.dma_start(out=attn_o[:, :], in_=attn_sb)

            # ---------------- context: attn columns + bf16 matvec ----------
            attn_bf = fin.tile([16, 2048], bf16, tag="attnbf")
            nc.vector.tensor_copy(attn_bf[:BPC, :T], attn_sb)
            pcols = fin.tile([128, 16, 16], bf16, tag="pcols")
            nc.sync.dma_start_transpose(pcols, attn_bf)
            for j in range(BPC):
                end_j = SUPER[nsb[j] - 1][0] + SUPER[nsb[j] - 1][1]
                ntb_j = (min(end_j, T) + 127) // 128
                pc = mi_ps.tile([1, 512], f32, tag="mi")
                for tb in range(ntb_j):
                    ptb = min(128, T - tb * 128)
                    si, tbl = tb // 4, tb % 4
                    nc.tensor.matmul(
                        pc, pcols[:ptb, tb, j:j + 1],
                        ebf[(j, si)][:ptb, tbl, :],
                        start=(tb == 0), stop=(tb == ntb_j - 1))
                cs = fin.tile([1, 512], f32, tag="cs", bufs=2)
                nc.vector.tensor_copy(cs, pc)
                ctxcol = fin.tile([128, 4], bf16, tag="ctxcol", bufs=2)
                for ke in range(4):
                    pst2 = tr_ps.tile([128, 128], f32, tag="tr")
                    nc.tensor.transpose(
                        pst2[:, :1], cs[:, ke * 128:(ke + 1) * 128],
                        identity[:1, :1])
                    nc.vector.tensor_copy(ctxcol[:, ke:ke + 1], pst2[:, :1])
                po = mi_ps.tile([1, 512], f32, tag="mi")
                for ke in range(4):
                    nc.tensor.matmul(
                        po, ctxcol[:, ke:ke + 1], wout_b[:, ke, :],
                        start=(ke == 0), stop=(ke == 3))
                o_sb = fin.tile([1, OUT], f32, tag="osb", bufs=2)
                nc.vector.tensor_add(o_sb, po, bout_s)
                nc.sync.dma_start(out=ctx_o[j:j + 1, :], in_=o_sb)

    return nc


def _get_nc(spec):
    key = (tuple(spec["nsb"]), tuple(spec["tw0"]), tuple(spec["twW"]),
           spec["Wmax"])
    if _built.get("key") != key:
        _apply_walrus_patches()
        _built["nc"] = _build_nc(spec)
        _built["key"] = key
    return _built["nc"]


def kernel(enc_states, enc_len, dec_states, W_enc, b_enc, W_dec, b_dec,
           conv_w, W_loc, b_loc, w_attn, W_out, b_out):
    from concourse.bass_utils import run_bass_kernel_spmd

    enc_states = np.ascontiguousarray(np.asarray(enc_states, dtype=np.float32))
    enc_len = np.asarray(enc_len, dtype=np.int32)
    dec_states = np.asarray(dec_states, dtype=np.float32)
    W_enc = np.asarray(W_enc, dtype=np.float32)
    b_enc = np.asarray(b_enc, dtype=np.float32)
    W_dec = np.asarray(W_dec, dtype=np.float32)
    b_dec = np.asarray(b_dec, dtype=np.float32)
    conv_w = np.asarray(conv_w, dtype=np.float32)
    W_loc = np.asarray(W_loc, dtype=np.float32)
    b_loc = np.asarray(b_loc, dtype=np.float32)
    w_attn = np.asarray(w_attn, dtype=np.float32)
    W_out = np.asarray(W_out, dtype=np.float32)
    b_out = np.asarray(b_out, dtype=np.float32)

    # ---- host-side weight transforms (all O(weights)) ----
    Wj = conv_w[:, 0, :].T @ W_loc                       # [2K+1, ATT]
    PW = np.concatenate([np.zeros((1, ATT), np.float64),
                         np.cumsum(Wj.astype(np.float64), 0)]).astype(np.float32)
    b_sum = b_enc + b_dec + b_loc
    pack_k = lambda w: np.ascontiguousarray(
        w.reshape(4, 128, -1).transpose(1, 0, 2))        # [128, 4, N]
    wenc_p = pack_k(W_enc)
    wdec_p = pack_k(W_dec)
    wout_p = pack_k(W_out)
    wat4_p = np.ascontiguousarray(w_attn.reshape(4, 128).T)
    bout_p = np.ascontiguousarray(b_out.reshape(1, OUT))

    # ---- L-sorted round-robin assignment: core c slot j <- rank j*8+c ----
    order = np.argsort(enc_len, kind="stable")
    assign = np.empty((N_CORES, BPC), np.int64)
    for j in range(BPC):
        for c in range(N_CORES):
            assign[c, j] = order[j * N_CORES + c]
    Ls = enc_len[assign]                                  # [cores, BPC]
    slotmax = Ls.max(axis=0)
    slotmin = Ls.min(axis=0)
    nsb = [max(1, sum(1 for (t0, _) in SUPER if t0 < int(slotmax[j])))
           for j in range(BPC)]
    tw0 = [int(slotmin[j]) - K for j in range(BPC)]
    twW = [int(slotmax[j]) - tw0[j] for j in range(BPC)]
    Wmax = max(twW)

    def pack_a(arr_ta):  # [W, ATT] -> [128, 4, W]
        return np.ascontiguousarray(
            arr_ta.T.reshape(4, 128, arr_ta.shape[0]).transpose(1, 0, 2))

    nc = _get_nc({"nsb": nsb, "tw0": tw0, "twW": twW, "Wmax": Wmax})

    in_maps = []
    for c in range(N_CORES):
        idx = assign[c]
        enc_c = np.ascontiguousarray(enc_states[idx])
        dec_c = dec_states[idx]
        dec4_p = np.ascontiguousarray(
            dec_c.T.reshape(4, 128, BPC).transpose(1, 0, 2))
        bias4_items = np.stack(
            [b_sum + PW[2 * K + 1] / float(enc_len[i]) for i in idx], 1)
        bias4_p = np.ascontiguousarray(
            bias4_items.reshape(4, 128, BPC).transpose(1, 0, 2))
        import ml_dtypes
        chead_p = np.empty((BPC, 128, 4, K), ml_dtypes.bfloat16)
        ctail_p = np.zeros((BPC, 128, 4, Wmax), ml_dtypes.bfloat16)
        for j in range(BPC):
            L = float(enc_len[idx[j]])
            t_head = np.arange(K)
            chead_p[j] = pack_a(-PW[K - t_head] / L)
            t_tail = tw0[j] + np.arange(twW[j])
            hi = np.clip(int(L) + K - t_tail, 0, 2 * K + 1)
            vals = (PW[hi] - PW[2 * K + 1]) / L
            vals[(t_tail < L - K) | (t_tail >= L)] = 0.0
            ctail_p[j, :, :, :twW[j]] = pack_a(vals.astype(np.float32))
        in_maps.append({
            "enc": enc_c,
            "encl4": np.ascontiguousarray(
                enc_len[idx].reshape(BPC, 1).astype(np.int32)),
            "wenc": wenc_p, "wdec": wdec_p, "wout": wout_p, "wat4": wat4_p,
            "dec4": dec4_p, "bias4": bias4_p,
            "chead": chead_p, "ctail": ctail_p, "boutd": bout_p,
        })

    res = run_bass_kernel_spmd(nc, in_maps, list(range(N_CORES)),
                               trace=bool(os.environ.get("KERNEL_TRACE")))
    _built["last_result"] = res
    context = np.empty((B, OUT), np.float32)
    attn = np.empty((B, T), np.float32)
    for c in range(N_CORES):
        context[assign[c]] = res.results[c]["ctx_o"]
        attn[assign[c]] = res.results[c]["attn_o"]
    return context, attn


# revision 18
# speedup vs baseline: 1.0799x; 1.0799x over previous
"""Location-aware attention on 8 Trainium2 NeuronCores (Bass/Tile).

Math (per batch item, reference semantics):
    enc_h = enc @ W_enc + b_enc                      [T, A]
    prev  = mask/L  (uniform attention over valid positions)
    loc   = conv1d(prev, conv_w).T @ W_loc + b_loc   [T, A]
    e     = tanh(enc_h + dec_h + loc) @ w_attn       [T]
    attn  = softmax(mask ? e : -inf)                 [T]
    ctx   = (attn @ enc) @ W_out + b_out             [O]

Kernel strategy (data-parallel over batch, 4 items/core):
  * prev is a step function, so conv1d(prev) is CONSTANT in t except within
    K=100 of the ends: loc(t) = PW[201]/L for t in [K, L-1-K], where PW is
    the prefix-sum over taps of Wj = conv_w^T @ W_loc (a host weight
    transform). The interior folds into the per-item tanh bias (together
    with dec_h and all biases); the head (t<K, static) and tail
    (t in [L-K, L), depends on L) get exact small correction tables added
    to the PSUM before the tanh.
  * enc_len is known when the kernel traces, so the program is specialized:
    items are L-sorted and dealt round-robin (core c, slot j <- rank j*8+c),
    which balances cores AND makes each slot's supertile count/edge windows
    near-uniform; work for t-tiles beyond a slot's max L is skipped.
  * The big GEMM runs in bf16 (measured 2.4e-3 rel err at K=512) in the
    [a-partitions, t-free] orientation: lhsT = W_enc k-tiles (native
    layout), rhs = enc^T built on-chip by PE transposes of the bf16 copy;
    enc itself is cast f32->bf16 inside the load DMA (SWDGE cast).
  * e = w_attn^T @ tanh via 1-column-stationary matvecs; e rows are staged
    through DRAM (partition-crossing move without SBUF->SBUF DMAs, which
    would serialize against the rest); masked softmax runs once on [4, T].
  * The bf16 enc copy stays resident in SBUF, so the context matvec needs
    no second HBM read; attention columns come from one xbar DMA transpose.
"""
import os

import numpy as np

B, T, ENC, DEC, ATT, CCH, OUT, K = 32, 2000, 512, 512, 512, 32, 512, 100
N_CORES = 8
BPC = B // N_CORES
SUPER = [(0, 512), (512, 512), (1024, 512), (1536, 464)]

_built = {}


# ---------------------------------------------------------------------------
# Walrus compat: this container's walrus build accepts at most ONE semaphore
# wait per instruction. (a) split the Tile exit-drain waits across several
# drains; (b) a to_json_bytes post-pass hoists excess waits onto
# EventSemaphore instructions inserted directly before the owner (same
# engine, so it blocks on the same condition one instruction earlier).
# ---------------------------------------------------------------------------
def _apply_walrus_patches():
    import orjson

    import concourse.bass as bass
    import concourse.mybir as mybir
    import concourse.tile as tile
    from concourse.vector_clock import ScopedClock

    if getattr(tile.TileContext, "_onewait_patched", False):
        return

    def _drain_and_barrier(self, tick_clock, wait_clock):
        nc = self.nc
        drain_inst = nc.sync.drain()
        wait_clock.add_sem_waits(
            drain_inst.ins, ScopedClock({None: tick_clock.global_clock})
        )
        ins = drain_inst.ins
        si = ins.sync_info
        waits = list(si.on_wait) if si and si.on_wait else []
        if len(waits) > 1:
            ins.sync_info = mybir.SyncInfo(
                on_wait=waits[:1], on_update=list(si.on_update or [])
            )
            for w in waits[1:]:
                d2 = nc.sync.drain()
                d2.ins.sync_info = mybir.SyncInfo(on_wait=[w], on_update=[])
        nc.all_engine_barrier()
        assert self.sems is not None
        popped = nc._tile_sem_poison_stack.pop()
        assert popped is self._sem_poison
        nc.clear_and_free_semaphores(list(self.sems.allocated().values()))
        nc.all_engine_barrier()

    _orig_to_json_bytes = bass.Bass.to_json_bytes
    counter = [0]

    def _split_to_json_bytes(self) -> bytes:
        j = orjson.loads(_orig_to_json_bytes(self))
        changed = False
        for f in j["functions"]:
            for bb in f["blocks"]:
                insts = bb.get("instructions") or []
                out = []
                for inst in insts:
                    si = inst.get("sync_info")
                    waits = (si or {}).get("on_wait") or []
                    if len(waits) > 1:
                        changed = True
                        for w in waits[:-1]:
                            counter[0] += 1
                            out.append({
                                "debug": inst.get("debug", 0),
                                "engine": inst["engine"],
                                "ins": [],
                                "outs": [],
                                "name": f"I-wsplit-{counter[0]}",
                                "opcode": "EventSemaphore",
                                "sync_info": {"on_update": [], "on_wait": [w]},
                            })
                        si["on_wait"] = [waits[-1]]
                    out.append(inst)
                bb["instructions"] = out
        return orjson.dumps(j) if changed else _orig_to_json_bytes(self)

    tile.TileContext._drain_and_barrier = _drain_and_barrier
    bass.Bass.to_json_bytes = _split_to_json_bytes
    tile.TileContext._onewait_patched = True


def _build_nc(spec):
    """spec: dict with static per-slot specialization:
    nsb[j]   - number of supertiles to compute for slot j
    tw0[j]   - tail-correction window start (col offset into ctail input)
    twW[j]   - tail window width (<= Wmax)
    Wmax     - ctail last-dim size
    """
    import concourse.bass as bass
    import concourse.mybir as mybir
    import concourse.tile as tile
    from concourse.masks import make_identity

    dt = mybir.dt
    f32, f32r, bf16, i32 = dt.float32, dt.float32r, dt.bfloat16, dt.int32
    Alu = mybir.AluOpType
    Act = mybir.ActivationFunctionType
    Axis = mybir.AxisListType

    nsb = spec["nsb"]
    tw0 = spec["tw0"]
    twW = spec["twW"]
    Wmax = spec["Wmax"]

    nc = bass.Bass()

    enc = nc.dram_tensor("enc", [BPC, T, ENC], f32, kind="ExternalInput")
    encl4 = nc.dram_tensor("encl4", [BPC, 1], i32, kind="ExternalInput")
    wenc = nc.dram_tensor("wenc", [128, 4, ATT], f32, kind="ExternalInput")
    wdec = nc.dram_tensor("wdec", [128, 4, ATT], f32, kind="ExternalInput")
    wout = nc.dram_tensor("wout", [128, 4, OUT], f32, kind="ExternalInput")
    wat4 = nc.dram_tensor("wat4", [128, 4], f32, kind="ExternalInput")
    dec4 = nc.dram_tensor("dec4", [128, 4, BPC], f32, kind="ExternalInput")
    bias4 = nc.dram_tensor("bias4", [128, 4, BPC], f32, kind="ExternalInput")
    chead = nc.dram_tensor("chead", [BPC, 128, 4, K], bf16, kind="ExternalInput")
    ctail = nc.dram_tensor("ctail", [BPC, 128, 4, Wmax], bf16,
                           kind="ExternalInput")
    boutd = nc.dram_tensor("boutd", [1, OUT], f32, kind="ExternalInput")
    e_dram = nc.dram_tensor("e_dram", [BPC, T], f32)
    ng_dram = nc.dram_tensor("ng_dram", [1, 1], f32)
    rs_dram = nc.dram_tensor("rs_dram", [BPC, 1], f32)
    ctx_o = nc.dram_tensor("ctx_o", [BPC, OUT], f32, kind="ExternalOutput")
    attn_o = nc.dram_tensor("attn_o", [BPC, T], f32, kind="ExternalOutput")

    with tile.TileContext(nc) as tc:
        with tc.tile_pool(name="const", bufs=1) as const, \
             tc.tile_pool(name="ebuf", bufs=1) as ebuf, \
             tc.tile_pool(name="et", bufs=3) as et_pool, \
             tc.tile_pool(name="th", bufs=6) as th_pool, \
             tc.tile_pool(name="ww", bufs=2) as ww_pool, \
             tc.tile_pool(name="fin", bufs=1) as fin, \
             tc.tile_pool(name="tr", bufs=4, space="PSUM") as tr_ps, \
             tc.tile_pool(name="xp", bufs=2, space="PSUM") as x_ps, \
             tc.tile_pool(name="mi", bufs=2, space="PSUM") as mi_ps:

            # ---------------- constants / weights -----------------
            identity_bf = const.tile([128, 128], bf16)
            make_identity(nc, identity_bf)
            identity = const.tile([128, 128], f32)
            make_identity(nc, identity)

            wenc_b = const.tile([128, 4, ATT], bf16)
            wout_b = const.tile([128, 4, OUT], bf16)
            wat4_b = const.tile([128, 4], bf16)
            base4 = const.tile([128, 4, BPC], f32)  # tanh bias per (a,m,item)
            bout_s = const.tile([1, OUT], f32)
            e_sb = const.tile([BPC, T], f32)
            gmax = const.tile([1, 512], f32)
            negmx4 = const.tile([BPC, 1], f32)
            elf = const.tile([BPC, 1], f32)
            ch_t = [const.tile([128, 4, K], bf16, name=f"ch_{j}")
                    for j in range(BPC)]
            ct_t = [const.tile([128, 4, Wmax], bf16, name=f"ct_{j}")
                    for j in range(BPC)]
            nc.vector.memset(e_sb, -30.0)
            nc.vector.memset(gmax, -1e30)
            # per-item edge-correction tables (a-major packing)
            for j in range(BPC):
                nc.sync.dma_start(out=ch_t[j], in_=chead[j, :, :, :])
                nc.sync.dma_start(out=ct_t[j], in_=ctail[j, :, :, :])

            nc.gpsimd.dma_start(out=wenc_b, in_=wenc[:, :, :])
            nc.gpsimd.dma_start(out=wout_b, in_=wout[:, :, :])
            nc.gpsimd.dma_start(out=wat4_b, in_=wat4[:, :])
            with tc.tile_pool(name="stage", bufs=1) as stage:
                nc.sync.dma_start(out=bout_s, in_=boutd[:, :])

                # base4[:, m, item] = dec_h^T + bias4 (bias4 already folds
                # b_enc+b_dec+b_loc and the interior conv response PW/L)
                s_wdec = stage.tile([128, 4, ATT], f32, tag="s8", name="s_wdec")
                nc.sync.dma_start(out=s_wdec, in_=wdec[:, :, :])
                s_dec4 = stage.tile([128, 4, BPC], f32, tag="s0b")
                nc.sync.dma_start(out=s_dec4, in_=dec4[:, :, :])
                s_bias4 = stage.tile([128, 4, BPC], f32, tag="s0c")
                nc.sync.dma_start(out=s_bias4, in_=bias4[:, :, :])
                for m in range(4):
                    ps_d = mi_ps.tile([128, BPC], f32, tag="mi")
                    for kd in range(4):
                        nc.tensor.matmul(
                            ps_d,
                            s_wdec[:, kd, m * 128:(m + 1) * 128],
                            s_dec4[:, kd, :],
                            start=(kd == 0), stop=(kd == 3),
                        )
                    nc.vector.tensor_add(base4[:, m, :], ps_d, s_bias4[:, m, :])

                s_el = stage.tile([BPC, 1], i32, tag="s0d")
                nc.sync.dma_start(out=s_el, in_=encl4[:, :])
                nc.vector.tensor_copy(elf, s_el)

            iota_f = fin.tile([BPC, T], f32, tag="iotaf")
            maskf = fin.tile([BPC, T], f32, tag="maskf")
            nc.gpsimd.iota(iota_f, pattern=[[1, T]], base=0,
                           channel_multiplier=0,
                           allow_small_or_imprecise_dtypes=True)
            nc.vector.tensor_scalar(
                out=maskf, in0=iota_f, scalar1=elf, scalar2=None, op0=Alu.is_lt)

            # ---------------- persistent bf16 enc copies -----------------
            ebf = {}
            for j in range(BPC):
                for si in range(nsb[j]):
                    ebf[(j, si)] = ebuf.tile(
                        [128, 4, ENC], bf16, tag=f"ebf_{j}_{si}",
                        name=f"ebf_{j}_{si}")

            def emit_transposes(j, si, et):
                t0, Nt = SUPER[si]
                for tb in range((Nt + 127) // 128):
                    ptb = min(128, Nt - tb * 128)
                    for ke in range(4):
                        pst = tr_ps.tile([128, 128], bf16, tag="tr")
                        nc.tensor.transpose(
                            pst[:, :ptb],
                            ebf[(j, si)][:ptb, tb, ke * 128:(ke + 1) * 128],
                            identity_bf[:ptb, :ptb])
                        nc.vector.tensor_copy(
                            et[:, ke, tb * 128: tb * 128 + ptb], pst[:, :ptb])

            # work list: (j, si) in supertile-major order
            work = [(j, si) for si in range(4) for j in range(BPC)
                    if si < nsb[j]]

            # batched cast-loads (SWDGE casts f32->bf16 during transfer)
            for si in range(4):
                for j in range(BPC):
                    if si >= nsb[j]:
                        continue
                    t0, Nt = SUPER[si]
                    for tb in range((Nt + 127) // 128):
                        ptb = min(128, Nt - tb * 128)
                        nc.gpsimd.dma_start(
                            out=ebf[(j, si)][:ptb, tb, :],
                            in_=enc[j, t0 + tb * 128: t0 + tb * 128 + ptb, :])

            # pipeline: transposes for the next work item are emitted before
            # this item's e-matvec so the PE never waits on the last tanh
            ets = {}
            ets[0] = et_pool.tile([128, 4, 512], bf16, tag="et", name="et_p0")
            emit_transposes(*work[0], ets[0])
            for w, (j, si) in enumerate(work):
                t0, Nt = SUPER[si]
                et = ets.pop(w)
                # ---- fused GEMM: enc_h^T (+ edge corrections), tanh ----
                ths = []
                xs = []
                for m in range(4):
                    xps = x_ps.tile([128, 512], f32, tag="x")
                    for ke in range(4):
                        nc.tensor.matmul(
                            xps[:, :Nt],
                            wenc_b[:, ke, m * 128:(m + 1) * 128],
                            et[:, ke, :Nt],
                            start=(ke == 0), stop=(ke == 3))
                    xs.append(xps)
                # edge corrections into PSUM (DVE), before tanh
                for m in range(4):
                    if si == 0:
                        nc.vector.tensor_add(
                            xs[m][:, :K], xs[m][:, :K], ch_t[j][:, m, :])
                    c0 = max(t0, tw0[j])
                    c1 = min(t0 + Nt, tw0[j] + twW[j])
                    if c0 < c1:
                        nc.vector.tensor_add(
                            xs[m][:, c0 - t0: c1 - t0],
                            xs[m][:, c0 - t0: c1 - t0],
                            ct_t[j][:, m, c0 - tw0[j]: c1 - tw0[j]])
                for m in range(4):
                    th = th_pool.tile([128, 512], bf16, tag="th")
                    nc.scalar.activation(
                        out=th[:, :Nt], in_=xs[m][:, :Nt], func=Act.Tanh,
                        bias=base4[:, m, j:j + 1], scale=1.0)
                    ths.append(th)
                # emit next work item's transposes ahead of the e-matvec
                if w + 1 < len(work):
                    ets[w + 1] = et_pool.tile([128, 4, 512], bf16, tag="et",
                                              name=f"et_p{w+1}")
                    emit_transposes(*work[w + 1], ets[w + 1])
                # ---- e = w_attn^T @ tanh -> [1, Nt] ----
                pe = mi_ps.tile([1, 512], f32, tag="mi")
                for m in range(4):
                    nc.tensor.matmul(
                        pe[:, :Nt], wat4_b[:, m:m + 1], ths[m][:, :Nt],
                        start=(m == 0), stop=(m == 3))
                etmp = ww_pool.tile([1, 512], f32, tag="etmp")
                nc.vector.tensor_copy(etmp[:, :Nt], pe[:, :Nt])
                nc.vector.tensor_max(
                    gmax[:, :Nt], gmax[:, :Nt], etmp[:, :Nt])
                # partition-crossing move staged via DRAM (no SBUF->SBUF DMA)
                nc.scalar.dma_start(
                    out=e_dram[j:j + 1, t0:t0 + Nt], in_=etmp[:, :Nt])

            # ---------------- masked softmax on [4, T] -----------------
            for j in range(BPC):
                end_j = SUPER[nsb[j] - 1][0] + SUPER[nsb[j] - 1][1]
                nc.scalar.dma_start(
                    out=e_sb[j:j + 1, :end_j], in_=e_dram[j:j + 1, :end_j])
            g1 = fin.tile([1, 1], f32, tag="g1")
            nc.vector.tensor_reduce(out=g1, in_=gmax, axis=Axis.X, op=Alu.max)
            ng1 = fin.tile([1, 1], f32, tag="ng1")
            nc.vector.tensor_scalar_mul(ng1, g1, -1.0)
            nc.scalar.dma_start(out=ng_dram[:, :], in_=ng1)
            nc.scalar.dma_start(out=negmx4, in_=bass.AP(
                ng_dram[:, :].tensor, 0, [[0, BPC], [1, 1]]))
            p_sb = fin.tile([BPC, T], f32, tag="p")
            nc.scalar.activation(
                out=p_sb, in_=e_sb, func=Act.Exp, bias=negmx4, scale=1.0)
            nc.vector.tensor_mul(p_sb, p_sb, maskf)
            # ctx path starts on UNNORMALIZED p; 1/sum is applied at the
            # output projection (softmax scale-invariance)
            attn_bf = fin.tile([16, 2048], bf16, tag="attnbf")
            nc.vector.tensor_copy(attn_bf[:BPC, :T], p_sb)
            pcols = fin.tile([128, 16, 16], bf16, tag="pcols")
            nc.sync.dma_start_transpose(pcols, attn_bf)
            ssum = fin.tile([BPC, 1], f32, tag="ssum")
            nc.vector.tensor_reduce(out=ssum, in_=p_sb, axis=Axis.X, op=Alu.add)
            rs = fin.tile([BPC, 1], f32, tag="rs")
            nc.vector.reciprocal(rs, ssum)
            nc.scalar.dma_start(out=rs_dram[:, :], in_=rs)
            nc.vector.tensor_scalar(
                out=p_sb, in0=p_sb, scalar1=rs, scalar2=None, op0=Alu.mult)
            attn_sb = p_sb
            nc.sync.dma_start(out=attn_o[:, :], in_=attn_sb)
            for j in range(BPC):
                end_j = SUPER[nsb[j] - 1][0] + SUPER[nsb[j] - 1][1]
                ntb_j = (min(end_j, T) + 127) // 128
                pc = mi_ps.tile([1, 512], f32, tag="mi")
                for tb in range(ntb_j):
                    ptb = min(128, T - tb * 128)
                    si, tbl = tb // 4, tb % 4
                    nc.tensor.matmul(
                        pc, pcols[:ptb, tb, j:j + 1],
                        ebf[(j, si)][:ptb, tbl, :],
                        start=(tb == 0), stop=(tb == ntb_j - 1))
                cs = fin.tile([1, 512], f32, tag="cs", bufs=2)
                nc.vector.tensor_copy(cs, pc)
                ctxcol = fin.tile([128, 4], bf16, tag="ctxcol", bufs=2)
                for ke in range(4):
                    pst2 = tr_ps.tile([128, 128], f32, tag="tr")
                    nc.tensor.transpose(
                        pst2[:, :1], cs[:, ke * 128:(ke + 1) * 128],
                        identity[:1, :1])
                    nc.vector.tensor_copy(ctxcol[:, ke:ke + 1], pst2[:, :1])
                po = mi_ps.tile([1, 512], f32, tag="mi")
                for ke in range(4):
                    nc.tensor.matmul(
                        po, ctxcol[:, ke:ke + 1], wout_b[:, ke, :],
                        start=(ke == 0), stop=(ke == 3))
                rs1 = fin.tile([1, 1], f32, tag="rs1", bufs=2)
                nc.scalar.dma_start(out=rs1, in_=rs_dram[j:j + 1, :])
                o_sb = fin.tile([1, OUT], f32, tag="osb", bufs=2)
                nc.vector.scalar_tensor_tensor(
                    out=o_sb, in0=po, scalar=rs1, in1=bout_s,
                    op0=Alu.mult, op1=Alu.add)
                nc.sync.dma_start(out=ctx_o[j:j + 1, :], in_=o_sb)

    return nc


def _get_nc(spec):
    key = (tuple(spec["nsb"]), tuple(spec["tw0"]), tuple(spec["twW"]),
           spec["Wmax"])
    if _built.get("key") != key:
        _apply_walrus_patches()
        _built["nc"] = _build_nc(spec)
        _built["key"] = key
    return _built["nc"]


def kernel(enc_states, enc_len, dec_states, W_enc, b_enc, W_dec, b_dec,
           conv_w, W_loc, b_loc, w_attn, W_out, b_out):
    from concourse.bass_utils import run_bass_kernel_spmd

    enc_states = np.ascontiguousarray(np.asarray(enc_states, dtype=np.float32))
    enc_len = np.asarray(enc_len, dtype=np.int32)
    dec_states = np.asarray(dec_states, dtype=np.float32)
    W_enc = np.asarray(W_enc, dtype=np.float32)
    b_enc = np.asarray(b_enc, dtype=np.float32)
    W_dec = np.asarray(W_dec, dtype=np.float32)
    b_dec = np.asarray(b_dec, dtype=np.float32)
    conv_w = np.asarray(conv_w, dtype=np.float32)
    W_loc = np.asarray(W_loc, dtype=np.float32)
    b_loc = np.asarray(b_loc, dtype=np.float32)
    w_attn = np.asarray(w_attn, dtype=np.float32)
    W_out = np.asarray(W_out, dtype=np.float32)
    b_out = np.asarray(b_out, dtype=np.float32)

    # ---- host-side weight transforms (all O(weights)) ----
    Wj = conv_w[:, 0, :].T @ W_loc                       # [2K+1, ATT]
    PW = np.concatenate([np.zeros((1, ATT), np.float64),
                         np.cumsum(Wj.astype(np.float64), 0)]).astype(np.float32)
    b_sum = b_enc + b_dec + b_loc
    pack_k = lambda w: np.ascontiguousarray(
        w.reshape(4, 128, -1).transpose(1, 0, 2))        # [128, 4, N]
    wenc_p = pack_k(W_enc)
    wdec_p = pack_k(W_dec)
    wout_p = pack_k(W_out)
    wat4_p = np.ascontiguousarray(w_attn.reshape(4, 128).T)
    bout_p = np.ascontiguousarray(b_out.reshape(1, OUT))

    # ---- L-sorted round-robin assignment: core c slot j <- rank j*8+c ----
    order = np.argsort(enc_len, kind="stable")
    assign = np.empty((N_CORES, BPC), np.int64)
    for j in range(BPC):
        for c in range(N_CORES):
            assign[c, j] = order[j * N_CORES + c]
    Ls = enc_len[assign]                                  # [cores, BPC]
    slotmax = Ls.max(axis=0)
    slotmin = Ls.min(axis=0)
    nsb = [max(1, sum(1 for (t0, _) in SUPER if t0 < int(slotmax[j])))
           for j in range(BPC)]
    tw0 = [int(slotmin[j]) - K for j in range(BPC)]
    twW = [int(slotmax[j]) - tw0[j] for j in range(BPC)]
    Wmax = max(twW)

    def pack_a(arr_ta):  # [W, ATT] -> [128, 4, W]
        return np.ascontiguousarray(
            arr_ta.T.reshape(4, 128, arr_ta.shape[0]).transpose(1, 0, 2))

    nc = _get_nc({"nsb": nsb, "tw0": tw0, "twW": twW, "Wmax": Wmax})

    in_maps = []
    for c in range(N_CORES):
        idx = assign[c]
        enc_c = np.ascontiguousarray(enc_states[idx])
        dec_c = dec_states[idx]
        dec4_p = np.ascontiguousarray(
            dec_c.T.reshape(4, 128, BPC).transpose(1, 0, 2))
        bias4_items = np.stack(
            [b_sum + PW[2 * K + 1] / float(enc_len[i]) for i in idx], 1)
        bias4_p = np.ascontiguousarray(
            bias4_items.reshape(4, 128, BPC).transpose(1, 0, 2))
        import ml_dtypes
        chead_p = np.empty((BPC, 128, 4, K), ml_dtypes.bfloat16)
        ctail_p = np.zeros((BPC, 128, 4, Wmax), ml_dtypes.bfloat16)
        for j in range(BPC):
            L = float(enc_len[idx[j]])
            t_head = np.arange(K)
            chead_p[j] = pack_a(-PW[K - t_head] / L)
            t_tail = tw0[j] + np.arange(twW[j])
            hi = np.clip(int(L) + K - t_tail, 0, 2 * K + 1)
            vals = (PW[hi] - PW[2 * K + 1]) / L
            vals[(t_tail < L - K) | (t_tail >= L)] = 0.0
            ctail_p[j, :, :, :twW[j]] = pack_a(vals.astype(np.float32))
        in_maps.append({
            "enc": enc_c,
            "encl4": np.ascontiguousarray(
                enc_len[idx].reshape(BPC, 1).astype(np.int32)),
            "wenc": wenc_p, "wdec": wdec_p, "wout": wout_p, "wat4": wat4_p,
            "dec4": dec4_p, "bias4": bias4_p,
            "chead": chead_p, "ctail": ctail_p, "boutd": bout_p,
        })

    res = run_bass_kernel_spmd(nc, in_maps, list(range(N_CORES)),
                               trace=bool(os.environ.get("KERNEL_TRACE")))
    _built["last_result"] = res
    context = np.empty((B, OUT), np.float32)
    attn = np.empty((B, T), np.float32)
    for c in range(N_CORES):
        context[assign[c]] = res.results[c]["ctx_o"]
        attn[assign[c]] = res.results[c]["attn_o"]
    return context, attn
